# revision 1
# baseline (speedup 1.0000x reference)
"""Block-tensorized linear (TT-factored block linear) on 8 Trainium2 NeuronCores.

Problem (hardcoded shapes):
    x:    (4, 2048, 4096) fp32   -> 8192 tokens, 4096 features
    U:    (4, 4, 1024, 256) fp32 (rows, cols, block, rank)
    V:    (4, 4, 256, 1024) fp32 (rows, cols, rank, block)
    bias: (4, 1024) fp32
    y[t, o*1024+m] = sum_c sum_r (sum_v x[t, c*1024+v] V[o,c,r,v]) U[o,c,m,r] + bias[o,m]

Sharding: 2-way tensor parallel over output row-block pairs (cores 0-3 take
o in {0,1}, cores 4-7 take o in {2,3}) x 4-way data parallel over tokens
(2048 tokens per core). Each core keeps its transposed U/V (16.8 MB) resident
in SBUF and streams its token shard.

Both matmul stages run in float32r (TF32-like single-pass mode, full PE rate
for moving dim >= 256; ~1.7e-4 end-to-end rel err vs ~2e-3 for bf16), with
tokens as the moving dimension so no on-device transposes are needed: the
host supplies x transposed (feature-major) and U/V pre-transposed per block.

Engine split: TensorE does both matmul stages back to back; VectorE rounds
stage-1 PSUM to f32r SBUF (CAST); ScalarE evicts stage-2 PSUM to SBUF so a
backed-up y-store DMA can never block the z-eviction path PE depends on.
The bias is a single broadcast add applied on the host during assembly.
"""

import numpy as np
from contextlib import ExitStack

NCORES = 8
TOK = 8192            # total tokens
D = 4096              # features
NB = 4                # num row/col blocks
BS = 1024             # block size
R = 256               # TT rank
TQ = TOK // 4         # tokens per core (2048)
TC = 512              # token chunk (moving dim for stage 1; 512 so the
                      # ~190ns f32r LDWEIGHTS hides under the ~213ns matmul)
NCHUNK = TQ // TC     # 4 chunks

_CACHE = {}


def _build_nc():
    if "nc" in _CACHE:
        return _CACHE["nc"]

    import concourse.bacc as bacc
    import concourse.tile as tile
    import concourse.mybir as mybir

    dt = mybir.dt
    f32r = dt.float32r

    nc = bacc.Bacc("TRN2", target_bir_lowering=False, debug=False)

    # x shard, host-tiled: [chunk, c, partition(v%128), vj*TC + t]
    xh_d = nc.dram_tensor("xh", [NCHUNK, NB, 128, 8 * TC], f32r, kind="ExternalInput").ap()
    # V^T per (o_loc, c) block: [b, p(v%128), vj*256 + r]
    vt_d = nc.dram_tensor("vt", [8, 128, 2048], f32r, kind="ExternalInput").ap()
    # U^T per (o_loc, c) block: [b, p(r%128), rj*1024 + m]
    ut_d = nc.dram_tensor("ut", [8, 128, 2048], f32r, kind="ExternalInput").ap()
    # output shard: [2048 tokens, o_loc*1024 + m] (bias added host-side)
    y_d = nc.dram_tensor("y", [TQ, 2048], dt.float32, kind="ExternalOutput").ap()

    with tile.TileContext(nc) as tcx, ExitStack() as ctx:
        wpool = ctx.enter_context(tcx.tile_pool(name="w", bufs=1))
        xpool = ctx.enter_context(tcx.tile_pool(name="xp", bufs=2))
        zpool = ctx.enter_context(tcx.tile_pool(name="zp", bufs=1))
        ypool = ctx.enter_context(tcx.tile_pool(name="yp", bufs=7))
        zps_pool = ctx.enter_context(tcx.tile_pool(name="zps", bufs=4, space="PSUM"))
        yps_pool = ctx.enter_context(tcx.tile_pool(name="yps", bufs=4, space="PSUM"))

        # resident weights, DMA'd in exactly the order the compute needs them:
        # V^T block for c / the c-th x slice / the o1 V^T block, interleaved
        # through chunk-0 stage 1; U^T halves arrive during chunk-0 compute.
        vtt = [None] * 8
        utt = [None] * 8

        def load_vt(b):
            # quarter DMAs into a bufs=1 tile: region-granular deps let the
            # vj=0 matmul start before the whole block has landed
            t = wpool.tile([128, 2048], f32r, tag=f"vt{b}")
            for q in range(4):
                nc.sync.dma_start(t[:, q * 512 : (q + 1) * 512], vt_d[b][:, q * 512 : (q + 1) * 512])
            vtt[b] = t

        def emit_stage1(tc_i):
            zsb = {}
            for c in range(NB):
                if tc_i == 0:
                    load_vt(c)  # o0 block for this c
                xc = xpool.tile([128, 8 * TC], f32r, tag="xc")
                if tc_i == 0 and c < 2:
                    # first use of each rotating slot: quarter DMAs are safe
                    # (no WAR on the slot) and start the PE ~4us earlier
                    for q in range(4):
                        cols = slice(q * 2 * TC, (q + 1) * 2 * TC)
                        nc.sync.dma_start(xc[:, cols], xh_d[tc_i, c, :, cols])
                else:
                    nc.sync.dma_start(xc[:], xh_d[tc_i, c])
                if tc_i == 0:
                    load_vt(4 + c)  # o1 block for this c
                for o in range(2):
                    b = o * 4 + c
                    for rj in range(2):
                        zps = zps_pool.tile([128, TC], dt.float32, tag="zps", name="zps")
                        for vj in range(8):
                            nc.tensor.matmul(
                                zps[:],
                                vtt[b][:, vj * 256 + rj * 128 : vj * 256 + rj * 128 + 128],
                                xc[:, vj * TC : (vj + 1) * TC],
                                start=(vj == 0),
                                stop=(vj == 7),
                            )
                        zt = zpool.tile([128, TC], f32r, tag=f"z{b}_{rj}")
                        nc.vector.tensor_copy(zt[:], zps[:])
                        zsb[(b, rj)] = zt
            return zsb

        def emit_stage2(tc_i, zsb):
            for o in range(2):
                for mc in range(2):
                    for tt in range(TC // 128):
                        yps = yps_pool.tile([128, 512], dt.float32, tag="yps", name="yps")
                        k = 0
                        for c in range(NB):
                            b = o * 4 + c
                            for rj in range(2):
                                nc.tensor.matmul(
                                    yps[:],
                                    zsb[(b, rj)][:, tt * 128 : (tt + 1) * 128],
                                    utt[b][:, rj * 1024 + mc * 512 : rj * 1024 + mc * 512 + 512],
                                    start=(k == 0),
                                    stop=(k == 7),
                                )
                                k += 1
                        ysb = ypool.tile([128, 512], dt.float32, tag="ysb")
                        nc.scalar.copy(ysb[:], yps[:])
                        t0 = tc_i * TC + tt * 128
                        # y stores go out on the SWDGE path so they never queue
                        # ahead of the next chunk's x loads in the HWDGE FIFOs;
                        # the last chunk switches to HWDGE (no loads left to
                        # contend with) to skip the ~6us SWDGE drain at the tail
                        eng = nc.sync if tc_i == NCHUNK - 1 else nc.gpsimd
                        eng.dma_start(
                            y_d[t0 : t0 + 128, o * 1024 + mc * 512 : o * 1024 + mc * 512 + 512],
                            ysb[:],
                        )

        for tc_i in range(NCHUNK):
            zsb = emit_stage1(tc_i)
            if tc_i == 0:
                # U^T in quarter-block DMAs ordered by first use in stage 2:
                # mc=0 halves for every block first, then mc=1 halves.
                for b in range(8):
                    utt[b] = wpool.tile([128, 2048], f32r, tag=f"ut{b}", name=f"ut{b}")
                for mc in range(2):
                    for b in range(8):
                        for rj in range(2):
                            cols = slice(rj * 1024 + mc * 512, rj * 1024 + mc * 512 + 512)
                            nc.sync.dma_start(utt[b][:, cols], ut_d[b][:, cols])
            emit_stage2(tc_i, zsb)

    nc.compile()
    _CACHE["nc"] = nc
    return nc


def _prep_in_maps(x, U, V, bias):
    x = np.ascontiguousarray(x, dtype=np.float32).reshape(TOK, D)
    U = np.asarray(U, dtype=np.float32)
    V = np.asarray(V, dtype=np.float32)

    # xh[tc, c, p, vj, tt] = x[tq*2048 + tc*TC + tt, c*1024 + vj*128 + p]
    xhs = []
    for tq in range(4):
        shard = x[tq * TQ : (tq + 1) * TQ]  # [2048, 4096]
        xh = shard.reshape(NCHUNK, TC, NB, 8, 128).transpose(0, 2, 4, 3, 1)
        xhs.append(np.ascontiguousarray(xh).reshape(NCHUNK, NB, 128, 8 * TC))

    vts, uts = [], []
    for og in range(2):
        Vg = V[og * 2 : og * 2 + 2]  # [2, 4, 256, 1024]
        vt = Vg.reshape(2, NB, 256, 8, 128).transpose(0, 1, 4, 3, 2)
        vts.append(np.ascontiguousarray(vt).reshape(8, 128, 2048))
        Ug = U[og * 2 : og * 2 + 2]  # [2, 4, 1024, 256]
        ut = Ug.reshape(2, NB, 1024, 2, 128).transpose(0, 1, 4, 3, 2)
        uts.append(np.ascontiguousarray(ut).reshape(8, 128, 2048))

    in_maps = []
    for g in range(NCORES):
        og, tq = g // 4, g % 4
        in_maps.append({"xh": xhs[tq], "vt": vts[og], "ut": uts[og]})
    return in_maps


def _assemble(results, bias):
    y = np.empty((TOK, D), dtype=np.float32)
    for g in range(NCORES):
        og, tq = g // 4, g % 4
        y[tq * TQ : (tq + 1) * TQ, og * 2048 : (og + 1) * 2048] = results[g]["y"]
    y = y.reshape(TOK // 2048, 2048, NB, BS)
    y += np.asarray(bias, dtype=np.float32)[None, None, :, :]
    return y.reshape(4, 2048, D)


def run_with_options(inputs, trace=False, **kw):
    from concourse.bass_utils import run_bass_kernel_spmd

    nc = _build_nc()
    in_maps = _prep_in_maps(**inputs)
    res = run_bass_kernel_spmd(nc, in_maps, core_ids=list(range(NCORES)), trace=trace, **kw)
    return _assemble(res.results, inputs["bias"]), res


def kernel(x, U, V, bias):
    out, _ = run_with_options({"x": x, "U": U, "V": V, "bias": bias})
    return out



# revision 2
# speedup vs baseline: 1.1375x; 1.1375x over previous
"""Block-tensorized linear (TT-factored block linear) on 8 Trainium2 NeuronCores.

Problem (hardcoded shapes):
    x:    (4, 2048, 4096) fp32   -> 8192 tokens, 4096 features
    U:    (4, 4, 1024, 256) fp32 (rows, cols, block, rank)
    V:    (4, 4, 256, 1024) fp32 (rows, cols, rank, block)
    bias: (4, 1024) fp32
    y[t, o*1024+m] = sum_c sum_r (sum_v x[t, c*1024+v] V[o,c,r,v]) U[o,c,m,r] + bias[o,m]

Sharding: 2-way tensor parallel over output row-block pairs (cores 0-3 take
o in {0,1}, cores 4-7 take o in {2,3}) x 4-way data parallel over tokens
(2048 tokens per core). Each core keeps its transposed U/V resident in SBUF
and streams its token shard.

All operands are bf16 (cast on host): the PE runs bf16 at the same 1 cycle/row
as f32r, but every DMA byte halves, which keeps chunk-0's front-loaded demand
(x chunk + all of V^T + all of U^T) under the ~330 GB/s HBM ceiling, and bf16
LDWEIGHTS (~107ns) hides fully under the 213ns matmuls where the f32r one
(~190ns) only barely did. End-to-end rel err ~2e-3 vs the 2e-2 gate.

Stage 1 streams vj (the contraction) OUTER over four concurrent PSUM
accumulation groups (o x rj), so the HBM demand is flat from the first matmul
instead of needing a whole V^T block + x slice in the first 8 matmuls.

Engine split: TensorE does both matmul stages back to back; VectorE rounds
stage-1 PSUM to bf16 SBUF (CAST); ScalarE evicts stage-2 PSUM to bf16 SBUF.
DMA queues: sync/HWDGE carries all weight loads (and last-chunk y stores),
scalar/HWDGE carries all x loads (dispatched in parallel with the first
weight pieces at startup; chunk i+1's x is prefetched early in stage 2 of
chunk i), gpsimd/SWDGE carries y stores for chunks 0-2 so a backed-up store
can never block a load ring. Bias is added on the host during assembly.
"""

import numpy as np
from contextlib import ExitStack

NCORES = 8
TOK = 8192            # total tokens
D = 4096              # features
NB = 4                # num row/col blocks
BS = 1024             # block size
R = 256               # TT rank
TQ = TOK // 4         # tokens per core (2048)
TC = 512              # token chunk (moving dim for stage 1)
NCHUNK = TQ // TC     # 4 chunks

_CACHE = {}


def _build_nc():
    if "nc" in _CACHE:
        return _CACHE["nc"]

    import concourse.bacc as bacc
    import concourse.tile as tile
    import concourse.mybir as mybir

    dt = mybir.dt
    bf16 = dt.bfloat16

    nc = bacc.Bacc("TRN2", target_bir_lowering=False, debug=False)

    # x shard, host-tiled: [chunk, c, partition(v%128), vj*TC + t]
    xh_d = nc.dram_tensor("xh", [NCHUNK, NB, 128, 8 * TC], bf16, kind="ExternalInput").ap()
    # V^T per (o_loc, c) block: [b, p(v%128), vj*256 + r]
    vt_d = nc.dram_tensor("vt", [8, 128, 2048], bf16, kind="ExternalInput").ap()
    # U^T per (o_loc, c) block: [b, p(r%128), rj*1024 + m]
    ut_d = nc.dram_tensor("ut", [8, 128, 2048], bf16, kind="ExternalInput").ap()
    # output shard: [2048 tokens, o_loc*1024 + m] (bias + f32 upcast host-side)
    y_d = nc.dram_tensor("y", [TQ, 2048], bf16, kind="ExternalOutput").ap()

    GROUPS = ((0, 0), (0, 1), (1, 0), (1, 1))  # (o, rj)

    with tile.TileContext(nc) as tcx, ExitStack() as ctx:
        wpool = ctx.enter_context(tcx.tile_pool(name="w", bufs=1))
        xpool = ctx.enter_context(tcx.tile_pool(name="xp", bufs=4))
        zpool = ctx.enter_context(tcx.tile_pool(name="zp", bufs=1))
        ypool = ctx.enter_context(tcx.tile_pool(name="yp", bufs=7))
        zps_pool = ctx.enter_context(tcx.tile_pool(name="zps", bufs=4, space="PSUM"))
        yps_pool = ctx.enter_context(tcx.tile_pool(name="yps", bufs=4, space="PSUM"))

        vtt = [None] * 8
        utt = [None] * 8
        xtiles = {}

        def load_vt_pair(c, npiece):
            # the o0/o1 blocks for this c, piece-interleaved on the sync ring
            # in exactly first-use order (vj-outer consumes cols left to right
            # in both blocks simultaneously)
            ta = wpool.tile([128, 2048], bf16, tag=f"vt{c}")
            tb = wpool.tile([128, 2048], bf16, tag=f"vt{4 + c}")
            w = 2048 // npiece
            for q in range(npiece):
                s = slice(q * w, (q + 1) * w)
                nc.sync.dma_start(ta[:, s], vt_d[c][:, s])
                nc.sync.dma_start(tb[:, s], vt_d[4 + c][:, s])
            vtt[c], vtt[4 + c] = ta, tb

        def emit_stage1(tc_i):
            zsb = {}
            for c in range(NB):
                if tc_i == 0:
                    # c=0 in fine pieces so the first matmul's deps are small
                    load_vt_pair(c, 4 if c == 0 else 2)
                    xc = xpool.tile([128, 8 * TC], bf16, tag="xc")
                    if c == 0:
                        for q in range(4):
                            cols = slice(q * 2 * TC, (q + 1) * 2 * TC)
                            nc.scalar.dma_start(xc[:, cols], xh_d[tc_i, c, :, cols])
                    else:
                        nc.scalar.dma_start(xc[:], xh_d[tc_i, c])
                else:
                    xc = xtiles.pop((tc_i, c))
                zps = {}
                for o, rj in GROUPS:
                    zps[(o, rj)] = zps_pool.tile([128, TC], dt.float32, tag="zps", name="zps")
                for vj in range(8):
                    for o, rj in GROUPS:
                        b = o * 4 + c
                        nc.tensor.matmul(
                            zps[(o, rj)][:],
                            vtt[b][:, vj * 256 + rj * 128 : vj * 256 + rj * 128 + 128],
                            xc[:, vj * TC : (vj + 1) * TC],
                            start=(vj == 0),
                            stop=(vj == 7),
                        )
                for o, rj in GROUPS:
                    b = o * 4 + c
                    zt = zpool.tile([128, TC], bf16, tag=f"z{b}_{rj}")
                    nc.vector.tensor_copy(zt[:], zps[(o, rj)][:])
                    zsb[(b, rj)] = zt
            return zsb

        def emit_stage2(tc_i, zsb):
            ev = 0
            for o in range(2):
                for mc in range(2):
                    for tt in range(TC // 128):
                        yps = yps_pool.tile([128, 512], dt.float32, tag="yps", name="yps")
                        k = 0
                        for c in range(NB):
                            b = o * 4 + c
                            for rj in range(2):
                                nc.tensor.matmul(
                                    yps[:],
                                    zsb[(b, rj)][:, tt * 128 : (tt + 1) * 128],
                                    utt[b][:, rj * 1024 + mc * 512 : rj * 1024 + mc * 512 + 512],
                                    start=(k == 0),
                                    stop=(k == 7),
                                )
                                k += 1
                        ysb = ypool.tile([128, 512], bf16, tag="ysb")
                        nc.scalar.copy(ysb[:], yps[:])
                        t0 = tc_i * TC + tt * 128
                        # y stores go out on the SWDGE path so they never queue
                        # ahead of loads in the HWDGE rings; the last chunk
                        # switches to the (by then idle) sync ring to skip the
                        # SWDGE drain at the tail
                        eng = nc.sync if tc_i == NCHUNK - 1 else nc.gpsimd
                        eng.dma_start(
                            y_d[t0 : t0 + 128, o * 1024 + mc * 512 : o * 1024 + mc * 512 + 512],
                            ysb[:],
                        )
                        ev += 1
                        if ev == 2 and tc_i < NCHUNK - 1:
                            # prefetch next chunk's x now: late enough that the
                            # transfers don't contend with chunk-0's U^T loads,
                            # early enough to land before stage 1 of chunk i+1
                            for c in range(NB):
                                xt = xpool.tile([128, 8 * TC], bf16, tag="xc")
                                nc.scalar.dma_start(xt[:], xh_d[tc_i + 1, c])
                                xtiles[(tc_i + 1, c)] = xt

        for tc_i in range(NCHUNK):
            zsb = emit_stage1(tc_i)
            if tc_i == 0:
                # U^T in 512-col pieces ordered by first use in stage 2:
                # (mc=0, rj=0/1) pieces for every block first, then mc=1.
                for b in range(8):
                    utt[b] = wpool.tile([128, 2048], bf16, tag=f"ut{b}", name=f"ut{b}")
                for mc in range(2):
                    for b in range(8):
                        for rj in range(2):
                            cols = slice(rj * 1024 + mc * 512, rj * 1024 + mc * 512 + 512)
                            nc.sync.dma_start(utt[b][:, cols], ut_d[b][:, cols])
            emit_stage2(tc_i, zsb)

    nc.compile()
    _CACHE["nc"] = nc
    return nc


def _prep_in_maps(x, U, V, bias):
    import ml_dtypes

    bf = ml_dtypes.bfloat16
    x = np.asarray(x, dtype=np.float32).reshape(TOK, D).astype(bf)
    U = np.asarray(U, dtype=np.float32).astype(bf)
    V = np.asarray(V, dtype=np.float32).astype(bf)

    # xh[tc, c, p, vj, tt] = x[tq*2048 + tc*TC + tt, c*1024 + vj*128 + p]
    xhs = []
    for tq in range(4):
        shard = x[tq * TQ : (tq + 1) * TQ]  # [2048, 4096]
        xh = shard.reshape(NCHUNK, TC, NB, 8, 128).transpose(0, 2, 4, 3, 1)
        xhs.append(np.ascontiguousarray(xh).reshape(NCHUNK, NB, 128, 8 * TC))

    vts, uts = [], []
    for og in range(2):
        Vg = V[og * 2 : og * 2 + 2]  # [2, 4, 256, 1024]
        vt = Vg.reshape(2, NB, 256, 8, 128).transpose(0, 1, 4, 3, 2)
        vts.append(np.ascontiguousarray(vt).reshape(8, 128, 2048))
        Ug = U[og * 2 : og * 2 + 2]  # [2, 4, 1024, 256]
        ut = Ug.reshape(2, NB, 1024, 2, 128).transpose(0, 1, 4, 3, 2)
        uts.append(np.ascontiguousarray(ut).reshape(8, 128, 2048))

    in_maps = []
    for g in range(NCORES):
        og, tq = g // 4, g % 4
        in_maps.append({"xh": xhs[tq], "vt": vts[og], "ut": uts[og]})
    return in_maps


def _assemble(results, bias):
    y = np.empty((TOK, D), dtype=np.float32)
    for g in range(NCORES):
        og, tq = g // 4, g % 4
        y[tq * TQ : (tq + 1) * TQ, og * 2048 : (og + 1) * 2048] = results[g]["y"].astype(
            np.float32
        )
    y = y.reshape(TOK // 2048, 2048, NB, BS)
    y += np.asarray(bias, dtype=np.float32)[None, None, :, :]
    return y.reshape(4, 2048, D)


def run_with_options(inputs, trace=False, **kw):
    from concourse.bass_utils import run_bass_kernel_spmd

    nc = _build_nc()
    in_maps = _prep_in_maps(**inputs)
    res = run_bass_kernel_spmd(nc, in_maps, core_ids=list(range(NCORES)), trace=trace, **kw)
    return _assemble(res.results, inputs["bias"]), res


def kernel(x, U, V, bias):
    out, _ = run_with_options({"x": x, "U": U, "V": V, "bias": bias})
    return out


# revision 4
# speedup vs baseline: 1.1476x; 1.0089x over previous
"""Block-tensorized linear (TT-factored block linear) on 8 Trainium2 NeuronCores.

Problem (hardcoded shapes):
    x:    (4, 2048, 4096) fp32   -> 8192 tokens, 4096 features
    U:    (4, 4, 1024, 256) fp32 (rows, cols, block, rank)
    V:    (4, 4, 256, 1024) fp32 (rows, cols, rank, block)
    bias: (4, 1024) fp32
    y[t, o*1024+m] = sum_c sum_r (sum_v x[t, c*1024+v] V[o,c,r,v]) U[o,c,m,r] + bias[o,m]

Sharding: 2-way tensor parallel over output row-block pairs (cores 0-3 take
o in {0,1}, cores 4-7 take o in {2,3}) x 4-way data parallel over tokens
(2048 tokens per core). Each core keeps its transposed U/V resident in SBUF
and streams its token shard in 2 chunks of 1024 tokens.

All operands are bf16 (cast on host): the PE runs bf16 at the same 1 cycle/row
as f32r, but every DMA byte halves, which keeps chunk-0's front-loaded demand
(x chunk + all of V^T + all of U^T) under the ~330 GB/s HBM ceiling, and bf16
LDWEIGHTS (~100ns) hides fully under the 213ns matmuls. Rel err ~4e-3 vs the
2e-2 gate. 1024-token chunks amortize the weight loads over twice the compute
window of 512-token chunks, which removes most chunk-0 DMA-wait stalls.

A short burst of dummy matmuls on a zeroed scratch tile runs during the ~8us
framework preamble + first-DMA window so the PE's HAM clock gate is already
at 2.4 GHz (it needs ~3.4us of sustained busy) when the first real matmul
issues; otherwise the first ~6 matmuls run at 1.2 GHz.

Stage 1 streams vj (the contraction) OUTER over four concurrent PSUM
accumulation groups (o x rj) per 512-token half, so the HBM demand is flat
from the first matmul. Engine split: TensorE does both matmul stages back to
back; VectorE rounds stage-1 PSUM to bf16 SBUF (CAST); ScalarE evicts stage-2
PSUM to bf16 SBUF. DMA queues: sync/HWDGE carries all weight loads (and
last-chunk y stores), scalar/HWDGE carries all x loads (first pieces sized
and ordered by first use), gpsimd/SWDGE carries chunk-0 y stores so a
backed-up store can never block a load ring. Bias is added on the host.
"""

import numpy as np
from contextlib import ExitStack

NCORES = 8
TOK = 8192            # total tokens
D = 4096              # features
NB = 4                # num row/col blocks
BS = 1024             # block size
R = 256               # TT rank
TQ = TOK // 4         # tokens per core (2048)
TC = 1024             # token chunk
NCHUNK = TQ // TC     # 2 chunks

_CACHE = {}


def _build_nc():
    if "nc" in _CACHE:
        return _CACHE["nc"]

    import concourse.bacc as bacc
    import concourse.tile as tile
    import concourse.mybir as mybir

    dt = mybir.dt
    bf16 = dt.bfloat16

    nc = bacc.Bacc("TRN2", target_bir_lowering=False, debug=False)

    # x shard, host-tiled: [chunk, c, partition(v%128), vj*TC + t]
    xh_d = nc.dram_tensor("xh", [NCHUNK, NB, 128, 8 * TC], bf16, kind="ExternalInput").ap()
    # V^T per (o_loc, c) block: [b, p(v%128), vj*256 + r]
    vt_d = nc.dram_tensor("vt", [8, 128, 2048], bf16, kind="ExternalInput").ap()
    # U^T per (o_loc, c) block: [b, p(r%128), rj*1024 + m]
    ut_d = nc.dram_tensor("ut", [8, 128, 2048], bf16, kind="ExternalInput").ap()
    # output shard: [2048 tokens, o_loc*1024 + m] (bias + f32 upcast host-side)
    y_d = nc.dram_tensor("y", [TQ, 2048], bf16, kind="ExternalOutput").ap()

    GROUPS = ((0, 0), (0, 1), (1, 0), (1, 1))  # (o, rj)

    # startup piece schedules (cols), ordered/sized by first use
    VT_PIECES0 = ((0, 256), (256, 512), (512, 1024), (1024, 1536), (1536, 2048))
    XC_PIECES0 = ((0, 512), (512, 1024), (1024, 2048), (2048, 4096), (4096, 6144), (6144, 8192))

    with tile.TileContext(nc) as tcx, ExitStack() as ctx:
        wpool = ctx.enter_context(tcx.tile_pool(name="w", bufs=1))
        xpool = ctx.enter_context(tcx.tile_pool(name="xp", bufs=4))
        zpool = ctx.enter_context(tcx.tile_pool(name="zp", bufs=1))
        ypool = ctx.enter_context(tcx.tile_pool(name="yp", bufs=7))
        warm_pool = ctx.enter_context(tcx.tile_pool(name="wm", bufs=1))
        zps_pool = ctx.enter_context(tcx.tile_pool(name="zps", bufs=4, space="PSUM"))
        yps_pool = ctx.enter_context(tcx.tile_pool(name="yps", bufs=4, space="PSUM"))

        # ---- PE warm-up: ~3.4us of dummy matmuls on zeroed scratch so the
        # HAM clock gate reaches 2.4 GHz before the first real matmul ----
        ws = warm_pool.tile([128, 512], bf16, tag="warm")
        nc.gpsimd.memset(ws[:], 0.0)
        wps = zps_pool.tile([128, 512], dt.float32, tag="zps", name="warm_ps")
        for _ in range(8):
            nc.tensor.matmul(wps[:], ws[:, 0:128], ws[:], start=True, stop=True)

        vtt = [None] * 8
        utt = [None] * 8
        xtiles = {}

        def load_vt_pair(c, pieces):
            # the o0/o1 blocks for this c, piece-interleaved on the sync ring
            # in first-use order (vj-outer consumes cols left to right in both
            # blocks simultaneously)
            ta = wpool.tile([128, 2048], bf16, tag=f"vt{c}")
            tb = wpool.tile([128, 2048], bf16, tag=f"vt{4 + c}")
            for lo, hi in pieces:
                s = slice(lo, hi)
                nc.sync.dma_start(ta[:, s], vt_d[c][:, s])
                nc.sync.dma_start(tb[:, s], vt_d[4 + c][:, s])
            vtt[c], vtt[4 + c] = ta, tb

        def emit_stage1(tc_i):
            zsb = {}
            for c in range(NB):
                if tc_i == 0:
                    load_vt_pair(c, VT_PIECES0 if c == 0 else ((0, 1024), (1024, 2048)))
                    xc = xpool.tile([128, 8 * TC], bf16, tag="xc")
                    xp = XC_PIECES0 if c == 0 else ((0, 4096), (4096, 8192))
                    for lo, hi in xp:
                        nc.scalar.dma_start(xc[:, lo:hi], xh_d[tc_i, c, :, lo:hi])
                else:
                    xc = xtiles.pop((tc_i, c))
                for th in range(2):
                    zps = {}
                    for g in GROUPS:
                        zps[g] = zps_pool.tile([128, 512], dt.float32, tag="zps", name="zps")
                    for vj in range(8):
                        for o, rj in GROUPS:
                            b = o * 4 + c
                            nc.tensor.matmul(
                                zps[(o, rj)][:],
                                vtt[b][:, vj * 256 + rj * 128 : vj * 256 + rj * 128 + 128],
                                xc[:, vj * TC + th * 512 : vj * TC + th * 512 + 512],
                                start=(vj == 0),
                                stop=(vj == 7),
                            )
                    for o, rj in GROUPS:
                        b = o * 4 + c
                        if th == 0:
                            zsb[(b, rj)] = zpool.tile(
                                [128, TC], bf16, tag=f"z{b}_{rj}", name=f"z{b}_{rj}"
                            )
                        nc.vector.tensor_copy(
                            zsb[(b, rj)][:, th * 512 : (th + 1) * 512], zps[(o, rj)][:]
                        )
            return zsb

        def emit_stage2(tc_i, zsb):
            ev = 0
            for o in range(2):
                for mc in range(2):
                    for tt in range(TC // 128):
                        yps = yps_pool.tile([128, 512], dt.float32, tag="yps", name="yps")
                        k = 0
                        for c in range(NB):
                            b = o * 4 + c
                            for rj in range(2):
                                nc.tensor.matmul(
                                    yps[:],
                                    zsb[(b, rj)][:, tt * 128 : (tt + 1) * 128],
                                    utt[b][:, rj * 1024 + mc * 512 : rj * 1024 + mc * 512 + 512],
                                    start=(k == 0),
                                    stop=(k == 7),
                                )
                                k += 1
                        ysb = ypool.tile([128, 512], bf16, tag="ysb")
                        nc.scalar.copy(ysb[:], yps[:])
                        t0 = tc_i * TC + tt * 128
                        # y stores go out on the SWDGE path so they never queue
                        # ahead of loads in the HWDGE rings; the last chunk
                        # switches to the (by then idle) sync ring to skip the
                        # SWDGE drain at the tail
                        eng = nc.sync if tc_i == NCHUNK - 1 else nc.gpsimd
                        eng.dma_start(
                            y_d[t0 : t0 + 128, o * 1024 + mc * 512 : o * 1024 + mc * 512 + 512],
                            ysb[:],
                        )
                        ev += 1
                        if ev == 2 and tc_i < NCHUNK - 1:
                            # prefetch next chunk's x now: late enough that the
                            # transfers don't contend with chunk-0's U^T loads,
                            # early enough to land before stage 1 of chunk i+1
                            for c in range(NB):
                                xt = xpool.tile([128, 8 * TC], bf16, tag="xc")
                                nc.scalar.dma_start(xt[:], xh_d[tc_i + 1, c])
                                xtiles[(tc_i + 1, c)] = xt

        for tc_i in range(NCHUNK):
            zsb = emit_stage1(tc_i)
            if tc_i == 0:
                # U^T in 512-col pieces ordered by first use in stage 2:
                # (mc=0, rj=0/1) pieces for every block first, then mc=1.
                for b in range(8):
                    utt[b] = wpool.tile([128, 2048], bf16, tag=f"ut{b}", name=f"ut{b}")
                for mc in range(2):
                    for b in range(8):
                        for rj in range(2):
                            cols = slice(rj * 1024 + mc * 512, rj * 1024 + mc * 512 + 512)
                            nc.sync.dma_start(utt[b][:, cols], ut_d[b][:, cols])
            emit_stage2(tc_i, zsb)

    nc.compile()
    _CACHE["nc"] = nc
    return nc


def _prep_in_maps(x, U, V, bias):
    import ml_dtypes

    bf = ml_dtypes.bfloat16
    x = np.asarray(x, dtype=np.float32).reshape(TOK, D).astype(bf)
    U = np.asarray(U, dtype=np.float32).astype(bf)
    V = np.asarray(V, dtype=np.float32).astype(bf)

    # xh[tc, c, p, vj, tt] = x[tq*2048 + tc*TC + tt, c*1024 + vj*128 + p]
    xhs = []
    for tq in range(4):
        shard = x[tq * TQ : (tq + 1) * TQ]  # [2048, 4096]
        xh = shard.reshape(NCHUNK, TC, NB, 8, 128).transpose(0, 2, 4, 3, 1)
        xhs.append(np.ascontiguousarray(xh).reshape(NCHUNK, NB, 128, 8 * TC))

    vts, uts = [], []
    for og in range(2):
        Vg = V[og * 2 : og * 2 + 2]  # [2, 4, 256, 1024]
        vt = Vg.reshape(2, NB, 256, 8, 128).transpose(0, 1, 4, 3, 2)
        vts.append(np.ascontiguousarray(vt).reshape(8, 128, 2048))
        Ug = U[og * 2 : og * 2 + 2]  # [2, 4, 1024, 256]
        ut = Ug.reshape(2, NB, 1024, 2, 128).transpose(0, 1, 4, 3, 2)
        uts.append(np.ascontiguousarray(ut).reshape(8, 128, 2048))

    in_maps = []
    for g in range(NCORES):
        og, tq = g // 4, g % 4
        in_maps.append({"xh": xhs[tq], "vt": vts[og], "ut": uts[og]})
    return in_maps


def _assemble(results, bias):
    y = np.empty((TOK, D), dtype=np.float32)
    for g in range(NCORES):
        og, tq = g // 4, g % 4
        y[tq * TQ : (tq + 1) * TQ, og * 2048 : (og + 1) * 2048] = results[g]["y"].astype(
            np.float32
        )
    y = y.reshape(TOK // 2048, 2048, NB, BS)
    y += np.asarray(bias, dtype=np.float32)[None, None, :, :]
    return y.reshape(4, 2048, D)


def run_with_options(inputs, trace=False, **kw):
    from concourse.bass_utils import run_bass_kernel_spmd

    nc = _build_nc()
    in_maps = _prep_in_maps(**inputs)
    res = run_bass_kernel_spmd(nc, in_maps, core_ids=list(range(NCORES)), trace=trace, **kw)
    return _assemble(res.results, inputs["bias"]), res


def kernel(x, U, V, bias):
    out, _ = run_with_options({"x": x, "U": U, "V": V, "bias": bias})
    return out
